# revision 31
# baseline (speedup 1.0000x reference)
"""Trainium2 Bass kernel for nn_Encoder_WordLstm (bi-LSTM over char/bichar embeddings).

Sharding: data-parallel over batch. Each of the 8 cores handles 8 sentences and
runs BOTH LSTM directions.

Scan layout (the serial bottleneck): the 1200 gate columns per side are split by
c-dim strip (0:128 padded to 172, 128:300) and re-ordered host-side into two
344-col PSUM banks per strip: bankA=[i|f], bankB=[g|o]. The four (side, strip)
matmul streams land on PE column-groups 0/32/64/96 and run concurrently; gates
for both strips and both sides share partition-spanning SBUF tiles so each
activation / elementwise instruction covers 4 row-strips at once while the two
side-chains stay independent (row slices 0:40 and 64:104). h/tanh(c) are bf16;
f*c runs on GPSIMD to unload the DVE. h is transposed back (row-tiled PE
transposes at partition bases 0/32/64/96) to feed the next step's lhsT.

Per-core pipeline:
  1. indirect-DMA gathers of 4 embedding streams x 2 sides -> feat [128tok, 800]
  2. PE transpose -> featT(bf16), matmul W_lin + tanh -> linT [300, 128tok] bf16
  3. matmul Wih (bias via ones-row) -> x tiles (bf16, padded gate layout) -> DRAM
  4. 512-step LSTM recurrence; right direction consumes pre-reversed gather
     indices so its scan is a plain forward loop.
Output hs [2, 512, 40, 172] bf16 per core; host reassembles [64, 512, 600].
"""

import os
import sys

import numpy as np

sys.path.insert(0, "/opt/trn_rl_repo")

import concourse.bass as bass
import concourse.bacc as bacc
import concourse.mybir as mybir
import concourse.tile as tile
from concourse.bass_utils import run_bass_kernel_spmd
from concourse.masks import make_identity

F32 = mybir.dt.float32
BF16 = mybir.dt.bfloat16
I32 = mybir.dt.int32
AF = mybir.ActivationFunctionType
ALU = mybir.AluOpType

B_TOT, S = 64, 512
DC = DB = 200
HID = H = 300
VC, VB = 10000, 200000
NCORES = 8
BL = B_TOT // NCORES          # 8 sentences per core
T = BL * S                    # 4096 tokens per core
G4 = 4 * H                    # 1200 true gate cols per side
GP = 1376                     # padded gate cols per side (4 banks x 344)
CW = 172                      # c-dims per strip slot (strip0 uses 128 + 44 pad)

N_TILES = T // 128            # 32

M300 = [128, 128, 44]         # chunks of 300 (lin output dims)
KXP = [128, 128, 65]          # xproj contraction chunks (65 = 44 dims + ones@64)


# scan strip row bases: (side, strip) -> psum/sbuf partition base
RBASE = {(0, 0): 0, (0, 1): 32, (1, 0): 64, (1, 1): 96}
# gate-col range in the padded-1376 layout for (strip, bank)
GCOL = {(0, 0): 0, (0, 1): 344, (1, 0): 688, (1, 1): 1032}


def _build_program():
    nc = bacc.Bacc()

    idx_d = nc.declare_dram_parameter("idx", [128, N_TILES * 8], I32, isOutput=False)
    tab_char = nc.declare_dram_parameter("char_embed", [VC, DC], F32, isOutput=False)
    tab_schar = nc.declare_dram_parameter("static_char_embed", [VC, DC], F32, isOutput=False)
    tab_bi = nc.declare_dram_parameter("bichar_embed", [VB, DB], F32, isOutput=False)
    tab_sbi = nc.declare_dram_parameter("static_bichar_embed", [VB, DB], F32, isOutput=False)
    wlin_d = nc.declare_dram_parameter("wlin_blk", [128, 24 * 128], BF16, isOutput=False)
    blin_d = nc.declare_dram_parameter("blin_blk", [128, 3], F32, isOutput=False)
    wih_d = nc.declare_dram_parameter("wihaug_blk", [128, 2 * 3 * GP], BF16, isOutput=False)
    whh12_d = nc.declare_dram_parameter("whh12_blk", [128, 2 * 2 * GP], BF16, isOutput=False)
    whh3_d = nc.declare_dram_parameter("whh3_blk", [44, 2 * GP], BF16, isOutput=False)
    i8_d = nc.declare_dram_parameter("i8blk", [8, 8], BF16, isOutput=False)
    ones_d = nc.declare_dram_parameter("onesblk", [1, 128], BF16, isOutput=False)
    hs_d = nc.declare_dram_parameter("hs", [2, S, 40, CW], BF16, isOutput=True)
    x_d = nc.dram_tensor("x_seq", [2, T, GP], BF16)

    tables = [tab_char, tab_schar, tab_bi, tab_sbi]

    with tile.TileContext(nc) as tc:
        with (
            tc.tile_pool(name="const", bufs=1) as cp,
            tc.tile_pool(name="ph_sb", bufs=2) as pp,
            tc.tile_pool(name="rc_sb", bufs=2) as rp,
            tc.tile_pool(name="ps", bufs=1, space="PSUM") as psp,
        ):
            ident = cp.tile([128, 128], F32, tag="ident")
            make_identity(nc, ident[:, :])
            identb = cp.tile([128, 128], BF16, tag="identb")
            nc.scalar.copy(identb[:, :], ident[:, :])
            idx_sb = cp.tile([128, N_TILES * 8], I32, tag="idx")
            nc.sync.dma_start(out=idx_sb[:, :], in_=idx_d[:, :])
            wlin_sb = cp.tile([128, 24 * 128], BF16, tag="wlin")
            nc.sync.dma_start(out=wlin_sb[:, :], in_=wlin_d[:, :])
            blin_sb = cp.tile([128, 3], F32, tag="blin")
            nc.sync.dma_start(out=blin_sb[:, :], in_=blin_d[:, :])
            wih_sb = cp.tile([128, 2 * 3 * GP], BF16, tag="wih")
            nc.sync.dma_start(out=wih_sb[:, :], in_=wih_d[:, :])
            whh12_sb = cp.tile([128, 2 * 2 * GP], BF16, tag="whh12")
            nc.sync.dma_start(out=whh12_sb[:, :], in_=whh12_d[:, :])

            # persistent linT tiles (side x parity); ones row 64 loaded once
            linTs = {}
            for side in range(2):
                for par in range(2):
                    lt = cp.tile([128, 3 * 128], BF16, tag=f"linT_{side}_{par}")
                    nc.sync.dma_start(out=lt[64:65, 256:384], in_=ones_d[:, :])
                    linTs[(side, par)] = lt

            # recurrence state: hT lhsT tiles per side, c shared [104, CW]
            hT12s, hT3s, b3s = [], [], []
            for c in range(2):
                t12 = cp.tile([128, 16], BF16, tag=f"hT12_{c}")
                nc.vector.memset(t12[:, :], 0.0)
                t3 = cp.tile([52, 8], BF16, tag=f"hT3_{c}")
                nc.vector.memset(t3[0:44, :], 0.0)
                nc.sync.dma_start(out=t3[44:52, 0:8], in_=i8_d[:, :])
                hT12s.append(t12); hT3s.append(t3)
                bufs = []
                for r in range(8):
                    b3 = cp.tile([52, GP], BF16, tag=f"b3_{c}_{r}")
                    nc.sync.dma_start(out=b3[0:44, :], in_=whh3_d[0:44, c * GP:(c + 1) * GP])
                    bufs.append(b3)
                b3s.append(bufs)
            c_st = cp.tile([104, CW], F32, tag="c_st")
            nc.vector.memset(c_st[:, :], 0.0)

            # ---------------- phases 1-3: gather, transpose, linear, xproj ----
            # Emitted as a function so phase tiles can interleave with scan
            # steps: they fill PE idle slots during the scan's dependency
            # stalls (keeps the HAM clock-gate warm).
            def emit_phase_part(t, side, stage):
                if stage == 0:
                    feat = pp.tile([128, 800], F32, tag=f"feat{side}")
                    for j4 in range(4):
                        col = t * 8 + side * 4 + j4
                        nc.gpsimd.indirect_dma_start(
                            out=feat[:, 200 * j4:200 * (j4 + 1)],
                            out_offset=None,
                            in_=tables[j4][:, :],
                            in_offset=bass.IndirectOffsetOnAxis(
                                ap=idx_sb[:, col:col + 1], axis=0),
                        )
                    # 8 transposes of 100-col slices (each inside one gather segment)
                    featT = pp.tile([128, 8 * 128], BF16, tag=f"ft{side}")
                    for kc in range(8):
                        tp = psp.tile([128, 128], F32, tag="px", bufs=2)
                        nc.tensor.transpose(
                            tp[0:100, 0:128], feat[:, kc * 100:(kc + 1) * 100],
                            ident[:, :])
                        nc.vector.tensor_copy(
                            featT[0:100, kc * 128:(kc + 1) * 128], tp[0:100, 0:128])
                    linT = linTs[(side, t % 2)]
                    for m in range(3):
                        mm = M300[m]
                        pl = psp.tile([128, 128], F32, tag="px", bufs=2)
                        for kc in range(8):
                            blk = (kc * 3 + m) * 128
                            nc.tensor.matmul(
                                pl[0:mm, 0:128],
                                lhsT=wlin_sb[0:100, blk:blk + mm],
                                rhs=featT[0:100, kc * 128:(kc + 1) * 128],
                                start=(kc == 0), stop=(kc == 7))
                        nc.scalar.activation(
                            linT[0:mm, m * 128:m * 128 + 128],
                            pl[0:mm, 0:128], AF.Tanh,
                            bias=blin_sb[0:mm, m:m + 1])
                else:
                    linT = linTs[(side, t % 2)]
                    x_sb = pp.tile([128, GP], BF16, tag=f"x{side}")
                    for q in range(4):
                        px = psp.tile([128, 344], F32, tag="px", bufs=2)
                        for kc in range(3):
                            kw = KXP[kc]
                            nc.tensor.matmul(
                                px[:, 0:344],
                                lhsT=linT[0:kw, kc * 128:kc * 128 + 128],
                                rhs=wih_sb[0:kw, side * 3 * GP + kc * GP + q * 344:
                                           side * 3 * GP + kc * GP + q * 344 + 344],
                                start=(kc == 0), stop=(kc == 2))
                        nc.scalar.copy(x_sb[:, q * 344:(q + 1) * 344], px[:, 0:344])
                    nc.sync.dma_start(
                        out=x_d[side, t * 128:(t + 1) * 128, :], in_=x_sb[:, :])

            # ---------------- phase 4: the two LSTM scans ---------------------
            # Per-side chains; gates for (side, strip) land on psum rows
            # 0:8 / 32:40 (side l) and 64:72 / 96:104 (side r) via col-tiling.
            def emit_scan_step(t):
                ga = psp.tile([104, 344], F32, tag="GA", bufs=2)
                gb = psp.tile([104, 344], F32, tag="GB", bufs=2)
                for side in range(2):
                    b3 = b3s[side][t % 8]
                    nc.sync.dma_start(
                        out=b3[44:52, :], in_=x_d[side, t * 8:(t + 1) * 8, :])
                    r0 = 64 * side
                    for bank, gt_ in ((0, ga), (1, gb)):
                        for strip in range(2):
                            rb = RBASE[(side, strip)]
                            col = GCOL[(strip, bank)]
                            # strip0 holds only 128 real cols per 172-slot;
                            # skip streaming the zero-pad columns
                            spans = ([(0, 128), (172, 128)] if strip == 0
                                     else [(0, 344)])
                            for (c0, cw) in spans:
                                out = gt_[rb:rb + 8, c0:c0 + cw]
                                nc.tensor.matmul(
                                    out, lhsT=hT12s[side][:, 0:8],
                                    rhs=whh12_sb[:, (side * 2) * GP + col + c0:
                                                 (side * 2) * GP + col + c0 + cw],
                                    start=True, stop=False, tile_position=(0, rb))
                                nc.tensor.matmul(
                                    out, lhsT=hT12s[side][:, 8:16],
                                    rhs=whh12_sb[:, (side * 2 + 1) * GP + col + c0:
                                                 (side * 2 + 1) * GP + col + c0 + cw],
                                    start=False, stop=False, tile_position=(0, rb))
                                nc.tensor.matmul(
                                    out, lhsT=hT3s[side][0:52, 0:8],
                                    rhs=b3[0:52, col + c0:col + c0 + cw],
                                    start=False, stop=True, tile_position=(0, rb))
                    # activations: rows r0:r0+40 cover both strips of this side.
                    # sigA + D=f*c run while the bankB matmuls still stream.
                    sga = rp.tile([104, 344], F32, tag="sga")
                    nc.scalar.activation(
                        sga[r0:r0 + 40, :], ga[r0:r0 + 40, :], AF.Sigmoid)
                    dt = rp.tile([104, CW], F32, tag="dt")
                    nc.vector.tensor_tensor(
                        dt[r0:r0 + 40, :], sga[r0:r0 + 40, CW:344],
                        c_st[r0:r0 + 40, :], op=ALU.mult)
                    gob = rp.tile([104, 344], F32, tag="gob")
                    nc.scalar.activation(
                        gob[r0:r0 + 40, 0:CW], gb[r0:r0 + 40, 0:CW], AF.Tanh)
                    nc.scalar.activation(
                        gob[r0:r0 + 40, CW:344], gb[r0:r0 + 40, CW:344], AF.Sigmoid)
                    # c' = f*c + i*g ; h = o*tanh(c')
                    pt = rp.tile([104, CW], F32, tag="pt")
                    nc.vector.tensor_tensor(
                        pt[r0:r0 + 40, :], sga[r0:r0 + 40, 0:CW],
                        gob[r0:r0 + 40, 0:CW], op=ALU.mult)
                    nc.vector.tensor_tensor(
                        c_st[r0:r0 + 40, :], pt[r0:r0 + 40, :],
                        dt[r0:r0 + 40, :], op=ALU.add)
                    tct = rp.tile([104, CW], F32, tag="tct")
                    nc.scalar.activation(
                        tct[r0:r0 + 40, :], c_st[r0:r0 + 40, :], AF.Tanh)
                    # h computed per strip so strip0's transpose (feeding the
                    # next step's first matmul pass) starts as early as possible
                    s0, s1 = r0, r0 + 32
                    h = rp.tile([104, CW], BF16, tag="h")
                    nc.vector.tensor_tensor(
                        h[s0:s0 + 8, :], gob[s0:s0 + 8, CW:344],
                        tct[s0:s0 + 8, :], op=ALU.mult)
                    tp1 = psp.tile([128, 8], BF16, tag="ptp", bufs=2)
                    nc.tensor.transpose(
                        tp1[0:128, 0:8], h[s0:s0 + 8, 0:128],
                        identb[s0:s0 + 8, s0:s0 + 8], tile_position=(s0, 0))
                    nc.scalar.copy(hT12s[side][:, 0:8], tp1[0:128, 0:8])
                    nc.vector.tensor_tensor(
                        h[s1:s1 + 8, :], gob[s1:s1 + 8, CW:344],
                        tct[s1:s1 + 8, :], op=ALU.mult)
                    tp2 = psp.tile([128, 8], BF16, tag="ptp", bufs=2)
                    nc.tensor.transpose(
                        tp2[0:128, 0:8], h[s1:s1 + 8, 0:128],
                        identb[s1:s1 + 8, s1:s1 + 8], tile_position=(s1, 0))
                    nc.scalar.copy(hT12s[side][:, 8:16], tp2[0:128, 0:8])
                    tp3 = psp.tile([128, 8], BF16, tag="ptp", bufs=2)
                    nc.tensor.transpose(
                        tp3[0:44, 0:8], h[s1:s1 + 8, 128:172],
                        identb[s1:s1 + 8, s1:s1 + 8], tile_position=(s1, 0))
                    nc.scalar.copy(hT3s[side][0:44, 0:8], tp3[0:44, 0:8])
                    nc.sync.dma_start(
                        out=hs_d[side, t, :, :], in_=h[r0:r0 + 40, :])

            # interleaved driver: phase tile k covers scan steps 16k..16k+16;
            # stay 3 tiles (48 steps) ahead of the scan's x consumption.
            # Quarter-tile granularity spreads the PE filler evenly.
            LEAD = 3
            for k in range(LEAD):
                for side in range(2):
                    for stage in range(2):
                        emit_phase_part(k, side, stage)
            for t in range(S):
                if t % 16 in (0, 4, 8, 12) and t // 16 + LEAD < N_TILES:
                    side, stage = divmod((t % 16) // 4, 2)
                    emit_phase_part(t // 16 + LEAD, side, stage)
                emit_scan_step(t)
    nc.compile()
    return nc


def _gate_perm():
    """Column permutation: padded-1376 col -> source col in reference (i,f,g,o)
    order, or -1 for a zero pad column."""
    perm = np.full(GP, -1, np.int64)
    goff = {"i": 0, "f": 300, "g": 600, "o": 900}
    order = {0: ("i", "f"), 1: ("g", "o")}
    for strip in range(2):
        lo, hi = (0, 128) if strip == 0 else (128, 300)
        for bank in range(2):
            base = GCOL[(strip, bank)]
            for slot, gname in enumerate(order[bank]):
                dst = base + slot * CW
                perm[dst:dst + (hi - lo)] = goff[gname] + np.arange(lo, hi)
    return perm


def _prep_host(inputs):
    """Build the per-core in_maps (host-side weight/index preprocessing)."""
    f = {k: np.asarray(v) for k, v in inputs.items()}

    wlinT = f["W_lin"].astype(np.float32).T            # [800, 300]
    wlin_blk = np.zeros((128, 24 * 128), np.float32)
    for kc in range(8):
        for m in range(3):
            mm = M300[m]
            blk = (kc * 3 + m) * 128
            wlin_blk[0:100, blk:blk + mm] = wlinT[kc * 100:(kc + 1) * 100,
                                                 m * 128:m * 128 + mm]
    blin_blk = np.zeros((128, 3), np.float32)
    for m in range(3):
        mm = M300[m]
        blin_blk[0:mm, m] = f["b_lin"][m * 128:m * 128 + mm]

    perm = _gate_perm()
    valid = perm >= 0

    wih_blk = np.zeros((128, 2 * 3 * GP), np.float32)
    whh12_blk = np.zeros((128, 2 * 2 * GP), np.float32)
    whh3_blk = np.zeros((44, 2 * GP), np.float32)
    for c, sfx in enumerate(("l", "r")):
        wihT = f[f"Wih_{sfx}"].astype(np.float32).T     # [300, 1200]
        bb = f[f"b_{sfx}"].astype(np.float32)           # [1200]
        wihP = np.zeros((300, GP), np.float32)
        wihP[:, valid] = wihT[:, perm[valid]]
        bbP = np.zeros(GP, np.float32)
        bbP[valid] = bb[perm[valid]]
        for kc in range(2):
            wih_blk[0:128, (c * 3 + kc) * GP:(c * 3 + kc + 1) * GP] = \
                wihP[kc * 128:(kc + 1) * 128, :]
        wih_blk[0:44, (c * 3 + 2) * GP:(c * 3 + 3) * GP] = wihP[256:300, :]
        wih_blk[64, (c * 3 + 2) * GP:(c * 3 + 3) * GP] = bbP
        whhT = f[f"Whh_{sfx}"].astype(np.float32).T     # [300, 1200]
        whhP = np.zeros((300, GP), np.float32)
        whhP[:, valid] = whhT[:, perm[valid]]
        whh12_blk[:, (c * 2) * GP:(c * 2 + 1) * GP] = whhP[0:128, :]
        whh12_blk[:, (c * 2 + 1) * GP:(c * 2 + 2) * GP] = whhP[128:256, :]
        whh3_blk[:, c * GP:(c + 1) * GP] = whhP[256:300, :]

    import ml_dtypes
    bf = lambda a: a.astype(ml_dtypes.bfloat16)
    shared = {
        "char_embed": f["char_embed"].astype(np.float32),
        "static_char_embed": f["static_char_embed"].astype(np.float32),
        "bichar_embed": f["bichar_embed"].astype(np.float32),
        "static_bichar_embed": f["static_bichar_embed"].astype(np.float32),
        "wlin_blk": bf(wlin_blk), "blin_blk": blin_blk,
        "wihaug_blk": bf(wih_blk), "whh12_blk": bf(whh12_blk),
        "whh3_blk": bf(whh3_blk),
        "i8blk": bf(np.eye(8, dtype=np.float32)),
        "onesblk": bf(np.ones((1, 128), np.float32)),
    }

    in_maps = []
    for core in range(NCORES):
        bs = slice(core * BL, (core + 1) * BL)
        idx_blk = np.zeros((128, N_TILES * 8), np.int32)
        # stream order: [charL scharL bilL sbilL | charR scharR birR sbirR]
        streams = [
            f["char_features"][bs].T.reshape(-1),
            f["static_char_features"][bs].T.reshape(-1),
            f["bichar_left_features"][bs].T.reshape(-1),
            f["static_bichar_left_features"][bs].T.reshape(-1),
            f["char_features"][bs][:, ::-1].T.reshape(-1),
            f["static_char_features"][bs][:, ::-1].T.reshape(-1),
            f["bichar_right_features"][bs][:, ::-1].T.reshape(-1),
            f["static_bichar_right_features"][bs][:, ::-1].T.reshape(-1),
        ]
        for t in range(N_TILES):
            for j in range(8):
                idx_blk[:, t * 8 + j] = streams[j][t * 128:(t + 1) * 128]
        in_maps.append({"idx": idx_blk, **shared})
    return in_maps


_CACHED = {}


def kernel(**inputs):
    if "nc" not in _CACHED:
        _CACHED["nc"] = _build_program()
    nc = _CACHED["nc"]
    in_maps = _prep_host(inputs)
    res = run_bass_kernel_spmd(nc, in_maps, list(range(NCORES)))
    _CACHED["res"] = res
    out = np.empty((B_TOT, S, 2 * H), np.float32)
    for core in range(NCORES):
        hs = res.results[core]["hs"].astype(np.float32)   # [2, S, 40, CW]
        bs = slice(core * BL, (core + 1) * BL)
        for side in range(2):
            hfull = np.empty((S, BL, H), np.float32)
            hfull[:, :, 0:128] = hs[side, :, 0:8, 0:128]
            hfull[:, :, 128:300] = hs[side, :, 32:40, 0:172]
            if side == 1:
                hfull = hfull[::-1]
            out[bs, :, side * H:(side + 1) * H] = hfull.transpose(1, 0, 2)
    return out


if __name__ == "__main__":
    sys.path.insert(0, os.path.dirname(os.path.abspath(__file__)))
    import reference
    inp = reference.setup_inputs()
    got = kernel(**{k: np.asarray(v) for k, v in inp.items()})
    exp = np.asarray(reference.reference(**inp))
    err = np.abs(got - exp)
    rel = err.max() / np.abs(exp).max()
    print("Relative error:", rel)
